# revision 1
# baseline (speedup 1.0000x reference)
"""Trainium2 Bass kernel for the two-branch GCN (nn_GCNN).

Math per branch (A includes self-loops and symmetric deg^-1/2 norm):
  S = A @ X                  (aggregate first: A @ (X @ W) == (A @ X) @ W)
  C = S @ W + b
  L = leaky_relu(C)
  pool^T[f, g] = sum_n L[n, f] * P[n, g] / cnt[g]
  h = leaky_relu(Wp^T @ pool + bp)        -> [128, 4] per core
head:
  hh = leaky_relu(Wf1^T @ [h1; h2] + bf1) -> [256, 4]
  h3 = leaky_relu(Wf2^T @ hh + bf2)       -> [64, 4]
  o  = sigmoid(Wo^T @ h3 + bo)            -> [1, 4]

Sharding across the 8 NeuronCores: 4 graphs per core; nodes and edges
are partitioned by the graph of the edge *destination*, so aggregation,
pooling and the per-branch MLP are fully core-local.  X (bf16) and all
parameters are replicated.  Edges are dst-sorted and packed into
128-edge subtiles; gathers use indirect DMA on the src index, and the
scatter-add is a one-hot (norm-valued) matmul accumulated in PSUM.
The final [1, 4] per-core outputs are concatenated on the host.
"""

import numpy as np
import ml_dtypes

import concourse.bacc as bacc
import concourse.mybir as mybir
import concourse.tile as tile
from concourse.bass import IndirectOffsetOnAxis
from concourse.bass_utils import run_bass_kernel_spmd
from concourse.masks import make_identity

BF16 = mybir.dt.bfloat16
F32 = mybir.dt.float32
I32 = mybir.dt.int32
P = 128
N_CORES = 8
N_GRAPHS = 32
GPC = N_GRAPHS // N_CORES  # graphs per core

DIMS = dict(n_nodes=10000, f_in=1024, fp=128, hf1=256, hf2=64)


# ---------------------------------------------------------------- host prep


def _branch_prep(x, edge_index, batch, n_nodes, f_in):
    """Per-branch host preprocessing. Returns per-core arrays + static meta."""
    src = np.asarray(edge_index[0], dtype=np.int64)
    dst = np.asarray(edge_index[1], dtype=np.int64)
    batch = np.asarray(batch, dtype=np.int64)

    deg = np.bincount(dst, minlength=n_nodes).astype(np.float64) + 1.0
    dinv = (1.0 / np.sqrt(deg)).astype(np.float32).astype(np.float64)

    # append self loops
    allsrc = np.concatenate([src, np.arange(n_nodes, dtype=np.int64)])
    alldst = np.concatenate([dst, np.arange(n_nodes, dtype=np.int64)])
    norm = (dinv[allsrc] * dinv[alldst]).astype(np.float32)

    # node ranges per core (batch is sorted)
    bounds = np.searchsorted(batch, np.arange(0, N_GRAPHS + 1, GPC))
    n_per_core = np.diff(bounds)
    npad = int(np.ceil(max(1, n_per_core.max()) / P) * P)
    t_d = npad // P

    edge_core = batch[alldst] // GPC

    # per (core, dtile) edge counts -> shared T_s[d]
    e_counts = np.zeros((N_CORES, t_d), dtype=np.int64)
    core_edges = []
    for c in range(N_CORES):
        m = edge_core == c
        es, ed, en = allsrc[m], alldst[m], norm[m]
        ld = ed - bounds[c]
        order = np.argsort(ld, kind="stable")
        es, ld, en = es[order], ld[order], en[order]
        core_edges.append((es, ld, en))
        e_counts[c] = np.bincount(ld // P, minlength=t_d)
    t_s = np.maximum(1, np.ceil(e_counts.max(axis=0) / P).astype(np.int64))
    t0 = np.concatenate([[0], np.cumsum(t_s)])
    t_tot = int(t0[-1])

    x_bf = np.ascontiguousarray(np.asarray(x, dtype=np.float32)).astype(
        ml_dtypes.bfloat16
    )

    per_core = []
    for c in range(N_CORES):
        es, ld, en = core_edges[c]
        src_arr = np.zeros((t_tot, P), dtype=np.int32)
        hot = np.zeros((t_tot, P, P), dtype=ml_dtypes.bfloat16)
        for d in range(t_d):
            m = (ld // P) == d
            k = int(m.sum())
            if k == 0:
                continue
            sl_src = es[m]
            sl_m = (ld[m] - d * P).astype(np.int64)
            sl_n = en[m]
            for j in range(int(t_s[d])):
                lo, hi = j * P, min((j + 1) * P, k)
                if lo >= k:
                    break
                t = int(t0[d]) + j
                nrows = hi - lo
                src_arr[t, :nrows] = sl_src[lo:hi]
                hot[t, np.arange(nrows), sl_m[lo:hi]] = sl_n[lo:hi].astype(
                    ml_dtypes.bfloat16
                )
        # pool matrix [t_d, P, GPC]; counts
        pm = np.zeros((t_d, P, GPC), dtype=ml_dtypes.bfloat16)
        nc_lo, nc_hi = bounds[c], bounds[c + 1]
        loc_g = (batch[nc_lo:nc_hi] - c * GPC).astype(np.int64)
        idx = np.arange(nc_hi - nc_lo)
        pm[idx // P, idx % P, loc_g] = 1.0
        cnt = np.bincount(loc_g, minlength=GPC).astype(np.float64)
        ci = (1.0 / np.maximum(cnt, 1.0)).astype(np.float32)
        per_core.append(
            {
                "src": src_arr,
                "hot": hot,
                "pm": pm,
                "ci": np.broadcast_to(
                    np.tile(ci, f_in // P), (P, (f_in // P) * GPC)
                ).copy(),
            }
        )
    meta = {"t_d": t_d, "t_s": [int(v) for v in t_s], "t0": [int(v) for v in t0]}
    return x_bf, per_core, meta


def _ktile(w, f_in):
    """[f_in, n] -> [P, (f_in//P)*n] SBUF k-tile layout."""
    f, n = w.shape
    assert f == f_in
    return (
        np.ascontiguousarray(w)
        .reshape(f // P, P, n)
        .transpose(1, 0, 2)
        .reshape(P, (f // P) * n)
    )


def prep_inputs(inputs, dims):
    n_nodes, f_in = dims["n_nodes"], dims["f_in"]
    fp, hf1, hf2 = dims["fp"], dims["hf1"], dims["hf2"]

    x1_bf, pc1, meta1 = _branch_prep(
        inputs["pro1_x"], inputs["pro1_edge_index"], inputs["pro1_batch"], n_nodes, f_in
    )
    x2_bf, pc2, meta2 = _branch_prep(
        inputs["pro2_x"], inputs["pro2_edge_index"], inputs["pro2_batch"], n_nodes, f_in
    )

    f32 = np.float32
    shared = {
        "xg1": x1_bf,
        "xg2": x2_bf,
        "wg1": _ktile(np.asarray(inputs["Wg1"], f32), f_in).astype(ml_dtypes.bfloat16),
        "wg2": _ktile(np.asarray(inputs["Wg2"], f32), f_in).astype(ml_dtypes.bfloat16),
        "bg1": np.asarray(inputs["bg1"], f32)[None, :].astype(ml_dtypes.bfloat16),
        "bg2": np.asarray(inputs["bg2"], f32)[None, :].astype(ml_dtypes.bfloat16),
        "wp1": _ktile(np.asarray(inputs["Wp1"], f32), f_in),
        "wp2": _ktile(np.asarray(inputs["Wp2"], f32), f_in),
        "bp1": np.asarray(inputs["bp1"], f32)[:, None],
        "bp2": np.asarray(inputs["bp2"], f32)[:, None],
        "wf1": _ktile(np.asarray(inputs["Wf1"], f32), 2 * fp),
        "bf1": np.asarray(inputs["bf1"], f32).reshape(hf1 // P, P).T.copy(),
        "wf2": _ktile(np.asarray(inputs["Wf2"], f32), hf1),
        "bf2": np.asarray(inputs["bf2"], f32)[:, None],
        "wo": np.asarray(inputs["Wo"], f32),
        "bo": np.asarray(inputs["bo"], f32)[:, None],
    }
    in_maps = []
    for c in range(N_CORES):
        m = dict(shared)
        for br, pc in (("1", pc1), ("2", pc2)):
            for k in ("src", "hot", "pm", "ci"):
                m[k + br] = pc[c][k]
        in_maps.append(m)
    meta = {"b1": meta1, "b2": meta2, "dims": dims}
    return in_maps, meta


# ---------------------------------------------------------------- program


def _bias_leaky(nc, pool, out_ap, psum_ap, bias_col):
    """out = leaky_relu(psum + bias); bias_col is a per-partition [p,1] AP."""
    p, n = psum_ap.shape
    z = pool.tile([p, n], F32, tag="blz")
    nc.vector.tensor_scalar_add(out=z[:], in0=psum_ap, scalar1=bias_col)
    t = pool.tile([p, n], F32, tag="blt")
    nc.vector.tensor_scalar_mul(out=t[:], in0=z[:], scalar1=0.01)
    nc.vector.tensor_tensor(
        out=out_ap, in0=z[:], in1=t[:], op=mybir.AluOpType.max
    )


def build_program(meta):
    dims = meta["dims"]
    n_nodes, f_in = dims["n_nodes"], dims["f_in"]
    fp, hf1, hf2 = dims["fp"], dims["hf1"], dims["hf2"]
    CH = f_in // P  # k-chunks of gcn layer
    NH = (f_in + 511) // 512  # N-halves of 512
    NS = min(f_in, 512)

    nc = bacc.Bacc("TRN2", target_bir_lowering=False, debug=False, num_devices=N_CORES)

    def din(name, shape, dt):
        return nc.dram_tensor(name, list(shape), dt, kind="ExternalInput").ap()

    aps = {}
    for br in ("1", "2"):
        m = meta["b" + br]
        t_tot = m["t0"][-1]
        aps["xg" + br] = din("xg" + br, [n_nodes, f_in], BF16)
        aps["src" + br] = din("src" + br, [t_tot, P], I32)
        aps["hot" + br] = din("hot" + br, [t_tot, P, P], BF16)
        aps["pm" + br] = din("pm" + br, [m["t_d"], P, GPC], BF16)
        aps["ci" + br] = din("ci" + br, [P, CH * GPC], F32)
        aps["wg" + br] = din("wg" + br, [P, CH * f_in], BF16)
        aps["bg" + br] = din("bg" + br, [1, f_in], BF16)
        aps["wp" + br] = din("wp" + br, [P, CH * fp], F32)
        aps["bp" + br] = din("bp" + br, [fp, 1], F32)
    aps["wf1"] = din("wf1", [P, (2 * fp // P) * hf1], F32)
    aps["bf1"] = din("bf1", [P, hf1 // P], F32)
    aps["wf2"] = din("wf2", [P, (hf1 // P) * hf2], F32)
    aps["bf2"] = din("bf2", [hf2, 1], F32)
    aps["wo"] = din("wo", [hf2, 1], F32)
    aps["bo"] = din("bo", [1, 1], F32)
    out_ap = nc.dram_tensor("out", [1, GPC], F32, kind="ExternalOutput").ap()

    SIG = mybir.ActivationFunctionType.Sigmoid

    with tile.TileContext(nc) as tc:
        with (
            tc.tile_pool(name="const", bufs=1) as cpool,
            tc.tile_pool(name="gp", bufs=6) as gpool,
            tc.tile_pool(name="hp", bufs=6) as hpool,
            tc.tile_pool(name="ip", bufs=6) as ipool,
            tc.tile_pool(name="sp", bufs=2) as spool,
            tc.tile_pool(name="tp", bufs=2) as tpool,
            tc.tile_pool(name="lp", bufs=2) as lpool,
            tc.tile_pool(name="acc", bufs=1) as apool,
            tc.tile_pool(name="spsum", bufs=2, space="PSUM") as spsum,
            tc.tile_pool(name="tpsum", bufs=1, space="PSUM") as tpsum,
            tc.tile_pool(name="cpsum", bufs=1, space="PSUM") as cpsum,
            tc.tile_pool(name="mpsum", bufs=1, space="PSUM") as mpsum,
        ):
            ident = cpool.tile([P, P], BF16)
            make_identity(nc, ident[:])
            ones1 = cpool.tile([1, P], BF16)
            nc.vector.memset(ones1[:], 1.0)

            # persistent weights
            wt = {}
            for name, dt in (
                ("wg1", BF16), ("wg2", BF16), ("bg1", BF16), ("bg2", BF16),
                ("wp1", F32), ("wp2", F32), ("bp1", F32), ("bp2", F32),
                ("ci1", F32), ("ci2", F32),
                ("wf1", F32), ("bf1", F32), ("wf2", F32), ("bf2", F32),
                ("wo", F32), ("bo", F32),
            ):
                t = cpool.tile(list(aps[name].shape), dt, tag=name)
                nc.sync.dma_start(out=t[:], in_=aps[name][:])
                wt[name] = t

            hbr = {}
            for br in ("1", "2"):
                m = meta["b" + br]
                t_d, t_s, t0 = m["t_d"], m["t_s"], m["t0"]
                xg, srca, hota, pma = (
                    aps["xg" + br], aps["src" + br], aps["hot" + br], aps["pm" + br]
                )
                poolacc = apool.tile([P, CH * GPC], F32, tag="poolacc" + br)
                nc.vector.memset(poolacc[:], 0.0)

                for d in range(t_d):
                    s_ps = spsum.tile([P, f_in], F32, tag="s")
                    for j in range(t_s[d]):
                        t = t0[d] + j
                        idxt = ipool.tile([P, 1], I32, tag="idx")
                        nc.sync.dma_start(out=idxt[:], in_=srca[t, :, None])
                        g = gpool.tile([P, f_in], BF16, tag="g")
                        nc.gpsimd.indirect_dma_start(
                            out=g[:],
                            out_offset=None,
                            in_=xg[:],
                            in_offset=IndirectOffsetOnAxis(ap=idxt[:, :1], axis=0),
                        )
                        hott = hpool.tile([P, P], BF16, tag="hot")
                        nc.sync.dma_start(out=hott[:], in_=hota[t])
                        for h in range(NH):
                            nc.tensor.matmul(
                                s_ps[:, h * NS : (h + 1) * NS],
                                lhsT=hott[:],
                                rhs=g[:, h * NS : (h + 1) * NS],
                                start=(j == 0),
                                stop=(j == t_s[d] - 1),
                            )
                    s_sb = spool.tile([P, f_in], BF16, tag="s_sb")
                    nc.scalar.copy(out=s_sb[:], in_=s_ps[:])
                    t_ps = tpsum.tile([P, f_in], BF16, tag="t_ps")
                    for ck in range(CH):
                        nc.tensor.transpose(
                            t_ps[:, ck * P : (ck + 1) * P],
                            s_sb[:, ck * P : (ck + 1) * P],
                            ident[:],
                        )
                    t_sb = tpool.tile([P, f_in], BF16, tag="t_sb")
                    nc.vector.tensor_copy(out=t_sb[:], in_=t_ps[:])

                    c_ps = cpsum.tile([P, f_in], F32, tag="c_ps")
                    for h in range(NH):
                        for kk in range(CH):
                            nc.tensor.matmul(
                                c_ps[:, h * NS : (h + 1) * NS],
                                lhsT=t_sb[:, kk * P : (kk + 1) * P],
                                rhs=wt["wg" + br][
                                    :, kk * f_in + h * NS : kk * f_in + (h + 1) * NS
                                ],
                                start=(kk == 0),
                                stop=False,
                            )
                        nc.tensor.matmul(
                            c_ps[:, h * NS : (h + 1) * NS],
                            lhsT=ones1[:1, :],
                            rhs=wt["bg" + br][:1, h * NS : (h + 1) * NS],
                            start=False,
                            stop=True,
                        )
                    leak = lpool.tile([P, f_in], BF16, tag="leak")
                    lk01 = lpool.tile([P, f_in], F32, tag="lk01")
                    nc.vector.tensor_scalar_mul(out=lk01[:], in0=c_ps[:], scalar1=0.01)
                    nc.vector.tensor_tensor(
                        out=leak[:], in0=c_ps[:], in1=lk01[:], op=mybir.AluOpType.max
                    )

                    pmt = hpool.tile([P, GPC], BF16, tag="pm")
                    nc.sync.dma_start(out=pmt[:], in_=pma[d])
                    p_ps = mpsum.tile([P, CH * GPC], F32, tag="small")
                    for ck in range(CH):
                        nc.tensor.matmul(
                            p_ps[:, ck * GPC : (ck + 1) * GPC],
                            lhsT=leak[:, ck * P : (ck + 1) * P],
                            rhs=pmt[:],
                            start=True,
                            stop=True,
                        )
                    nc.vector.tensor_add(out=poolacc[:], in0=poolacc[:], in1=p_ps[:])

                # scale by 1/cnt, then h = lrelu(Wp^T @ pool + bp)
                nc.vector.tensor_tensor(
                    out=poolacc[:],
                    in0=poolacc[:],
                    in1=wt["ci" + br][:],
                    op=mybir.AluOpType.mult,
                )
                h_ps = mpsum.tile([P, GPC], F32, tag="small")
                for ck in range(CH):
                    nc.tensor.matmul(
                        h_ps[:, :],
                        lhsT=wt["wp" + br][:, ck * fp : (ck + 1) * fp],
                        rhs=poolacc[:, ck * GPC : (ck + 1) * GPC],
                        start=(ck == 0),
                        stop=(ck == CH - 1),
                    )
                hb = apool.tile([fp, GPC], F32, tag="hbr" + br)
                _bias_leaky(nc, apool, hb[:], h_ps[:fp, :], wt["bp" + br][:, :1])
                hbr[br] = hb

            # head
            K1 = 2 * fp // P
            M1 = hf1 // P
            rhs_k = [hbr["1"], hbr["2"]]
            hh = apool.tile([P, M1 * GPC], F32, tag="hh")
            for mt in range(M1):
                f_ps = mpsum.tile([P, GPC], F32, tag="small")
                for kk in range(K1):
                    nc.tensor.matmul(
                        f_ps[:, :],
                        lhsT=wt["wf1"][:, kk * hf1 + mt * P : kk * hf1 + (mt + 1) * P],
                        rhs=rhs_k[kk][:, :],
                        start=(kk == 0),
                        stop=(kk == K1 - 1),
                    )
                _bias_leaky(
                    nc, apool, hh[:, mt * GPC : (mt + 1) * GPC], f_ps[:, :],
                    wt["bf1"][:, mt : mt + 1],
                )
            g_ps = mpsum.tile([hf2, GPC], F32, tag="small")
            for kk in range(M1):
                nc.tensor.matmul(
                    g_ps[:, :],
                    lhsT=wt["wf2"][:, kk * hf2 : (kk + 1) * hf2],
                    rhs=hh[:, kk * GPC : (kk + 1) * GPC],
                    start=(kk == 0),
                    stop=(kk == M1 - 1),
                )
            h3 = apool.tile([hf2, GPC], F32, tag="h3")
            _bias_leaky(nc, apool, h3[:], g_ps[:], wt["bf2"][:, :1])
            o_ps = mpsum.tile([1, GPC], F32, tag="small")
            nc.tensor.matmul(
                o_ps[:, :], lhsT=wt["wo"][:, :1], rhs=h3[:, :], start=True, stop=True
            )
            o_sb = apool.tile([1, GPC], F32, tag="o_sb")
            nc.scalar.activation(
                out=o_sb[:], in_=o_ps[:], func=SIG, bias=wt["bo"][:1, :1]
            )
            nc.sync.dma_start(out=out_ap[:], in_=o_sb[:])

    nc.compile()
    return nc


# ---------------------------------------------------------------- entry


_CACHE = {}


def _program_key(meta):
    return (
        tuple(meta["b1"]["t_s"]),
        tuple(meta["b2"]["t_s"]),
        meta["b1"]["t_d"],
        meta["b2"]["t_d"],
    )


def get_program(meta):
    key = _program_key(meta)
    if key not in _CACHE:
        _CACHE[key] = build_program(meta)
    return _CACHE[key]


def kernel(**inputs) -> np.ndarray:
    in_maps, meta = prep_inputs(inputs, DIMS)
    nc = get_program(meta)
    res = run_bass_kernel_spmd(nc, in_maps, core_ids=list(range(N_CORES)))
    out = np.concatenate(
        [np.asarray(res.results[c]["out"], dtype=np.float32).reshape(GPC) for c in range(N_CORES)]
    )
    return out[:, None]


# revision 4
# speedup vs baseline: 114.2135x; 114.2135x over previous
"""Trainium2 Bass kernel for the two-branch GCN (nn_GCNN).

Math per branch (A includes self-loops and symmetric deg^-1/2 norm):
  S = A @ X                  (aggregate first: A @ (X @ W) == (A @ X) @ W)
  C = S @ W + b
  L = leaky_relu(C)
  pool^T[f, g] = sum_n L[n, f] * P[n, g] / cnt[g]
  h = leaky_relu(Wp^T @ pool + bp)        -> [128, 4] per core
head:
  hh = leaky_relu(Wf1^T @ [h1; h2] + bf1) -> [256, 4]
  h3 = leaky_relu(Wf2^T @ hh + bf2)       -> [64, 4]
  o  = sigmoid(Wo^T @ h3 + bo)            -> [1, 4]

Sharding across the 8 NeuronCores: 4 graphs per core; nodes and edges
are partitioned by the graph of the edge *destination*, so aggregation,
pooling and the per-branch MLP are fully core-local.  X (bf16) and all
parameters are replicated.  Edges are dst-sorted and packed into
128-edge subtiles; gathers use indirect DMA on the src index, and the
scatter-add is a one-hot (norm-valued) matmul accumulated in PSUM.
The final [1, 4] per-core outputs are concatenated on the host.
"""

import contextlib

import numpy as np
import ml_dtypes

import concourse.bacc as bacc
import concourse.mybir as mybir
import concourse.tile as tile
from concourse.bass import IndirectOffsetOnAxis
from concourse.bass_utils import run_bass_kernel_spmd
from concourse.masks import make_identity

BF16 = mybir.dt.bfloat16
F32 = mybir.dt.float32
I32 = mybir.dt.int32
P = 128
N_CORES = 8
N_GRAPHS = 32
GPC = N_GRAPHS // N_CORES  # graphs per core

DIMS = dict(n_nodes=10000, f_in=1024, fp=128, hf1=256, hf2=64)


# ---------------------------------------------------------------- host prep


def _branch_prep(x, edge_index, batch, n_nodes, f_in):
    """Per-branch host preprocessing. Returns per-core arrays + static meta."""
    src = np.asarray(edge_index[0], dtype=np.int64)
    dst = np.asarray(edge_index[1], dtype=np.int64)
    batch = np.asarray(batch, dtype=np.int64)

    deg = np.bincount(dst, minlength=n_nodes).astype(np.float64) + 1.0
    dinv = (1.0 / np.sqrt(deg)).astype(np.float32).astype(np.float64)

    # append self loops
    allsrc = np.concatenate([src, np.arange(n_nodes, dtype=np.int64)])
    alldst = np.concatenate([dst, np.arange(n_nodes, dtype=np.int64)])
    norm = (dinv[allsrc] * dinv[alldst]).astype(np.float32)

    # node ranges per core (batch is sorted)
    bounds = np.searchsorted(batch, np.arange(0, N_GRAPHS + 1, GPC))
    n_per_core = np.diff(bounds)
    npad = int(np.ceil(max(1, n_per_core.max()) / P) * P)
    t_d = npad // P

    edge_core = batch[alldst] // GPC

    # per (core, dtile) edge counts -> shared T_s[d]
    e_counts = np.zeros((N_CORES, t_d), dtype=np.int64)
    core_edges = []
    for c in range(N_CORES):
        m = edge_core == c
        es, ed, en = allsrc[m], alldst[m], norm[m]
        ld = ed - bounds[c]
        order = np.argsort(ld, kind="stable")
        es, ld, en = es[order], ld[order], en[order]
        core_edges.append((es, ld, en))
        e_counts[c] = np.bincount(ld // P, minlength=t_d)
    t_s = np.maximum(1, np.ceil(e_counts.max(axis=0) / P).astype(np.int64))
    t0 = np.concatenate([[0], np.cumsum(t_s)])
    t_tot = int(t0[-1])

    x_bf = np.ascontiguousarray(np.asarray(x, dtype=np.float32)).astype(
        ml_dtypes.bfloat16
    )

    per_core = []
    for c in range(N_CORES):
        es, ld, en = core_edges[c]
        src_arr = np.zeros((t_tot, P), dtype=np.int32)
        hot = np.zeros((t_tot, P, P), dtype=ml_dtypes.bfloat16)
        for d in range(t_d):
            m = (ld // P) == d
            k = int(m.sum())
            if k == 0:
                continue
            sl_src = es[m]
            sl_m = (ld[m] - d * P).astype(np.int64)
            sl_n = en[m]
            for j in range(int(t_s[d])):
                lo, hi = j * P, min((j + 1) * P, k)
                if lo >= k:
                    break
                t = int(t0[d]) + j
                nrows = hi - lo
                src_arr[t, :nrows] = sl_src[lo:hi]
                hot[t, np.arange(nrows), sl_m[lo:hi]] = sl_n[lo:hi].astype(
                    ml_dtypes.bfloat16
                )
        # pool matrix [t_d, P, GPC]; counts
        pm = np.zeros((t_d, P, GPC), dtype=ml_dtypes.bfloat16)
        nc_lo, nc_hi = bounds[c], bounds[c + 1]
        loc_g = (batch[nc_lo:nc_hi] - c * GPC).astype(np.int64)
        idx = np.arange(nc_hi - nc_lo)
        pm[idx // P, idx % P, loc_g] = 1.0
        cnt = np.bincount(loc_g, minlength=GPC).astype(np.float64)
        ci = (1.0 / np.maximum(cnt, 1.0)).astype(np.float32)
        per_core.append(
            {
                "src": src_arr,
                "hot": hot,
                "pm": pm,
                "ci": np.broadcast_to(
                    np.tile(ci, f_in // P), (P, (f_in // P) * GPC)
                ).copy(),
            }
        )
    meta = {"t_d": t_d, "t_s": [int(v) for v in t_s], "t0": [int(v) for v in t0]}
    return x_bf, per_core, meta


def _ktile(w, f_in):
    """[f_in, n] -> [P, (f_in//P)*n] SBUF k-tile layout."""
    f, n = w.shape
    assert f == f_in
    return (
        np.ascontiguousarray(w)
        .reshape(f // P, P, n)
        .transpose(1, 0, 2)
        .reshape(P, (f // P) * n)
    )


def prep_inputs(inputs, dims):
    n_nodes, f_in = dims["n_nodes"], dims["f_in"]
    fp, hf1, hf2 = dims["fp"], dims["hf1"], dims["hf2"]

    x1_bf, pc1, meta1 = _branch_prep(
        inputs["pro1_x"], inputs["pro1_edge_index"], inputs["pro1_batch"], n_nodes, f_in
    )
    x2_bf, pc2, meta2 = _branch_prep(
        inputs["pro2_x"], inputs["pro2_edge_index"], inputs["pro2_batch"], n_nodes, f_in
    )

    f32 = np.float32
    shared = {
        "xg1": x1_bf,
        "xg2": x2_bf,
        "wg1": _ktile(np.asarray(inputs["Wg1"], f32), f_in).astype(ml_dtypes.bfloat16),
        "wg2": _ktile(np.asarray(inputs["Wg2"], f32), f_in).astype(ml_dtypes.bfloat16),
        "bg1": np.asarray(inputs["bg1"], f32)[None, :].astype(ml_dtypes.bfloat16),
        "bg2": np.asarray(inputs["bg2"], f32)[None, :].astype(ml_dtypes.bfloat16),
        "wp1": _ktile(np.asarray(inputs["Wp1"], f32), f_in),
        "wp2": _ktile(np.asarray(inputs["Wp2"], f32), f_in),
        "bp1": np.asarray(inputs["bp1"], f32)[:, None],
        "bp2": np.asarray(inputs["bp2"], f32)[:, None],
        "wf1": _ktile(np.asarray(inputs["Wf1"], f32), 2 * fp),
        "bf1": np.asarray(inputs["bf1"], f32).reshape(hf1 // P, P).T.copy(),
        "wf2": _ktile(np.asarray(inputs["Wf2"], f32), hf1),
        "bf2": np.asarray(inputs["bf2"], f32)[:, None],
        "wo": np.asarray(inputs["Wo"], f32),
        "bo": np.asarray(inputs["bo"], f32)[:, None],
    }
    in_maps = []
    for c in range(N_CORES):
        m = dict(shared)
        for br, pc in (("1", pc1), ("2", pc2)):
            for k in ("src", "hot", "pm", "ci"):
                m[k + br] = pc[c][k]
        in_maps.append(m)
    meta = {"b1": meta1, "b2": meta2, "dims": dims}
    return in_maps, meta


# ---------------------------------------------------------------- program


def _bias_leaky(nc, pool, out_ap, psum_ap, bias_col):
    """out = leaky_relu(psum + bias); bias_col is a per-partition [p,1] AP."""
    p, n = psum_ap.shape
    z = pool.tile([p, n], F32, tag="blz")
    nc.vector.tensor_scalar_add(out=z[:], in0=psum_ap, scalar1=bias_col)
    t = pool.tile([p, n], F32, tag="blt")
    nc.vector.tensor_scalar_mul(out=t[:], in0=z[:], scalar1=0.01)
    nc.vector.tensor_tensor(out=out_ap, in0=z[:], in1=t[:], op=mybir.AluOpType.max)


def build_program(meta, loop_n=1):
    dims = meta["dims"]
    n_nodes, f_in = dims["n_nodes"], dims["f_in"]
    fp, hf1, hf2 = dims["fp"], dims["hf1"], dims["hf2"]
    CH = f_in // P  # k-chunks of gcn layer
    NH = (f_in + 511) // 512  # N-halves of 512
    NS = min(f_in, 512)

    nc = bacc.Bacc("TRN2", target_bir_lowering=False, debug=False, num_devices=N_CORES)

    def din(name, shape, dt):
        return nc.dram_tensor(name, list(shape), dt, kind="ExternalInput").ap()

    aps = {}
    for br in ("1", "2"):
        m = meta["b" + br]
        t_tot = m["t0"][-1]
        aps["xg" + br] = din("xg" + br, [n_nodes, f_in], BF16)
        aps["src" + br] = din("src" + br, [t_tot, P], I32)
        aps["hot" + br] = din("hot" + br, [t_tot, P, P], BF16)
        aps["pm" + br] = din("pm" + br, [m["t_d"], P, GPC], BF16)
        aps["ci" + br] = din("ci" + br, [P, CH * GPC], F32)
        aps["wg" + br] = din("wg" + br, [P, CH * f_in], BF16)
        aps["bg" + br] = din("bg" + br, [1, f_in], BF16)
        aps["wp" + br] = din("wp" + br, [P, CH * fp], F32)
        aps["bp" + br] = din("bp" + br, [fp, 1], F32)
    aps["wf1"] = din("wf1", [P, (2 * fp // P) * hf1], F32)
    aps["bf1"] = din("bf1", [P, hf1 // P], F32)
    aps["wf2"] = din("wf2", [P, (hf1 // P) * hf2], F32)
    aps["bf2"] = din("bf2", [hf2, 1], F32)
    aps["wo"] = din("wo", [hf2, 1], F32)
    aps["bo"] = din("bo", [1, 1], F32)
    out_ap = nc.dram_tensor("out", [1, GPC], F32, kind="ExternalOutput").ap()

    SIG = mybir.ActivationFunctionType.Sigmoid

    with tile.TileContext(nc) as tc:
        with (
            tc.tile_pool(name="const", bufs=1) as cpool,
            tc.tile_pool(name="gp", bufs=6) as gpool,
            tc.tile_pool(name="hp", bufs=6) as hpool,
            tc.tile_pool(name="ip", bufs=6) as ipool,
            tc.tile_pool(name="sp", bufs=2) as spool,
            tc.tile_pool(name="tp", bufs=2) as tpool,
            tc.tile_pool(name="lp", bufs=2) as lpool,
            tc.tile_pool(name="acc", bufs=1) as apool,
            tc.tile_pool(name="spsum", bufs=2, space="PSUM") as spsum,
            tc.tile_pool(name="tpsum", bufs=1, space="PSUM") as tpsum,
            tc.tile_pool(name="cpsum", bufs=1, space="PSUM") as cpsum,
            tc.tile_pool(name="mpsum", bufs=1, space="PSUM") as mpsum,
        ):
            ident = cpool.tile([P, P], BF16)
            make_identity(nc, ident[:])
            ones1 = cpool.tile([1, P], BF16)
            nc.vector.memset(ones1[:], 1.0)

            # persistent weights
            wt = {}
            for name, dt in (
                ("wg1", BF16), ("wg2", BF16), ("bg1", BF16), ("bg2", BF16),
                ("wp1", F32), ("wp2", F32), ("bp1", F32), ("bp2", F32),
                ("ci1", F32), ("ci2", F32),
                ("wf1", F32), ("bf1", F32), ("wf2", F32), ("bf2", F32),
                ("wo", F32), ("bo", F32),
            ):
                t = cpool.tile(list(aps[name].shape), dt, tag=name)
                nc.sync.dma_start(out=t[:], in_=aps[name][:])
                wt[name] = t

            def emit_body():
                hbr = {}
                for br in ("1", "2"):
                    m = meta["b" + br]
                    t_d, t_s, t0 = m["t_d"], m["t_s"], m["t0"]
                    xg, srca, hota, pma = (
                        aps["xg" + br], aps["src" + br], aps["hot" + br],
                        aps["pm" + br],
                    )
                    poolacc = apool.tile([P, CH * GPC], F32, tag="poolacc" + br)
                    nc.vector.memset(poolacc[:], 0.0)

                    for d in range(t_d):
                        s_ps = spsum.tile([P, f_in], F32, tag="s")
                        for j in range(t_s[d]):
                            t = t0[d] + j
                            idxt = ipool.tile([P, 1], I32, tag="idx")
                            nc.sync.dma_start(out=idxt[:], in_=srca[t, :, None])
                            g = gpool.tile([P, f_in], BF16, tag="g")
                            nc.gpsimd.indirect_dma_start(
                                out=g[:],
                                out_offset=None,
                                in_=xg[:],
                                in_offset=IndirectOffsetOnAxis(ap=idxt[:, :1], axis=0),
                            )
                            hott = hpool.tile([P, P], BF16, tag="hot")
                            nc.sync.dma_start(out=hott[:], in_=hota[t])
                            for h in range(NH):
                                nc.tensor.matmul(
                                    s_ps[:, h * NS : (h + 1) * NS],
                                    lhsT=hott[:],
                                    rhs=g[:, h * NS : (h + 1) * NS],
                                    start=(j == 0),
                                    stop=(j == t_s[d] - 1),
                                )
                        s_sb = spool.tile([P, f_in], BF16, tag="s_sb")
                        nc.scalar.copy(out=s_sb[:], in_=s_ps[:])
                        t_ps = tpsum.tile([P, f_in], BF16, tag="t_ps")
                        for ck in range(CH):
                            nc.tensor.transpose(
                                t_ps[:, ck * P : (ck + 1) * P],
                                s_sb[:, ck * P : (ck + 1) * P],
                                ident[:],
                            )
                        t_sb = tpool.tile([P, f_in], BF16, tag="t_sb")
                        nc.vector.tensor_copy(out=t_sb[:], in_=t_ps[:])

                        c_ps = cpsum.tile([P, f_in], F32, tag="c_ps")
                        for h in range(NH):
                            for kk in range(CH):
                                nc.tensor.matmul(
                                    c_ps[:, h * NS : (h + 1) * NS],
                                    lhsT=t_sb[:, kk * P : (kk + 1) * P],
                                    rhs=wt["wg" + br][
                                        :, kk * f_in + h * NS : kk * f_in + (h + 1) * NS
                                    ],
                                    start=(kk == 0),
                                    stop=False,
                                )
                            nc.tensor.matmul(
                                c_ps[:, h * NS : (h + 1) * NS],
                                lhsT=ones1[:1, :],
                                rhs=wt["bg" + br][:1, h * NS : (h + 1) * NS],
                                start=False,
                                stop=True,
                            )
                        leak = lpool.tile([P, f_in], BF16, tag="leak")
                        lk01 = lpool.tile([P, f_in], F32, tag="lk01")
                        nc.vector.tensor_scalar_mul(
                            out=lk01[:], in0=c_ps[:], scalar1=0.01
                        )
                        nc.vector.tensor_tensor(
                            out=leak[:], in0=c_ps[:], in1=lk01[:],
                            op=mybir.AluOpType.max,
                        )

                        pmt = hpool.tile([P, GPC], BF16, tag="pm")
                        nc.sync.dma_start(out=pmt[:], in_=pma[d])
                        p_ps = mpsum.tile([P, CH * GPC], F32, tag="small")
                        for ck in range(CH):
                            nc.tensor.matmul(
                                p_ps[:, ck * GPC : (ck + 1) * GPC],
                                lhsT=leak[:, ck * P : (ck + 1) * P],
                                rhs=pmt[:],
                                start=True,
                                stop=True,
                            )
                        nc.vector.tensor_add(
                            out=poolacc[:], in0=poolacc[:], in1=p_ps[:]
                        )

                    # scale by 1/cnt, then h = lrelu(Wp^T @ pool + bp)
                    nc.vector.tensor_tensor(
                        out=poolacc[:],
                        in0=poolacc[:],
                        in1=wt["ci" + br][:],
                        op=mybir.AluOpType.mult,
                    )
                    h_ps = mpsum.tile([P, GPC], F32, tag="small")
                    for ck in range(CH):
                        nc.tensor.matmul(
                            h_ps[:, :],
                            lhsT=wt["wp" + br][:, ck * fp : (ck + 1) * fp],
                            rhs=poolacc[:, ck * GPC : (ck + 1) * GPC],
                            start=(ck == 0),
                            stop=(ck == CH - 1),
                        )
                    hb = apool.tile([fp, GPC], F32, tag="hbr" + br)
                    _bias_leaky(nc, apool, hb[:], h_ps[:fp, :], wt["bp" + br][:, :1])
                    hbr[br] = hb

                # head
                K1 = 2 * fp // P
                M1 = hf1 // P
                rhs_k = [hbr["1"], hbr["2"]]
                hh = apool.tile([P, M1 * GPC], F32, tag="hh")
                for mt in range(M1):
                    f_ps = mpsum.tile([P, GPC], F32, tag="small")
                    for kk in range(K1):
                        nc.tensor.matmul(
                            f_ps[:, :],
                            lhsT=wt["wf1"][
                                :, kk * hf1 + mt * P : kk * hf1 + (mt + 1) * P
                            ],
                            rhs=rhs_k[kk][:, :],
                            start=(kk == 0),
                            stop=(kk == K1 - 1),
                        )
                    _bias_leaky(
                        nc, apool, hh[:, mt * GPC : (mt + 1) * GPC], f_ps[:, :],
                        wt["bf1"][:, mt : mt + 1],
                    )
                g_ps = mpsum.tile([hf2, GPC], F32, tag="small")
                for kk in range(M1):
                    nc.tensor.matmul(
                        g_ps[:, :],
                        lhsT=wt["wf2"][:, kk * hf2 : (kk + 1) * hf2],
                        rhs=hh[:, kk * GPC : (kk + 1) * GPC],
                        start=(kk == 0),
                        stop=(kk == M1 - 1),
                    )
                h3 = apool.tile([hf2, GPC], F32, tag="h3")
                _bias_leaky(nc, apool, h3[:], g_ps[:], wt["bf2"][:, :1])
                o_ps = mpsum.tile([1, GPC], F32, tag="small")
                nc.tensor.matmul(
                    o_ps[:, :], lhsT=wt["wo"][:, :1], rhs=h3[:, :],
                    start=True, stop=True,
                )
                o_sb = apool.tile([1, GPC], F32, tag="o_sb")
                nc.scalar.activation(
                    out=o_sb[:], in_=o_ps[:], func=SIG, bias=wt["bo"][:1, :1]
                )
                nc.sync.dma_start(out=out_ap[:], in_=o_sb[:])

            if loop_n > 1:
                with tc.For_i(0, loop_n, 1):
                    emit_body()
            else:
                emit_body()

    nc.compile()
    return nc


# ---------------------------------------------------------------- entry


_CACHE = {}


def _program_key(meta):
    return (
        tuple(meta["b1"]["t_s"]),
        tuple(meta["b2"]["t_s"]),
        meta["b1"]["t_d"],
        meta["b2"]["t_d"],
    )


def get_program(meta):
    key = _program_key(meta)
    if key not in _CACHE:
        _CACHE[key] = build_program(meta)
    return _CACHE[key]


def kernel(**inputs) -> np.ndarray:
    in_maps, meta = prep_inputs(inputs, DIMS)
    nc = get_program(meta)
    res = run_bass_kernel_spmd(nc, in_maps, core_ids=list(range(N_CORES)))
    out = np.concatenate(
        [
            np.asarray(res.results[c]["out"], dtype=np.float32).reshape(GPC)
            for c in range(N_CORES)
        ]
    )
    return out[:, None]


# revision 12
# speedup vs baseline: 256.0075x; 2.2415x over previous
"""Trainium2 Bass kernel for the two-branch GCN (nn_GCNN).

Math per branch (A includes self-loops and symmetric deg^-1/2 norm):
  S = A @ X                  (aggregate first: A @ (X @ W) == (A @ X) @ W)
  C = S @ W + b
  L = leaky_relu(C)
  pool^T[f, g] = sum_n L[n, f] * P[n, g] / cnt[g]
  h = leaky_relu(Wp^T @ pool + bp)        -> [128, 4] per core
head:
  hh = leaky_relu(Wf1^T @ [h1; h2] + bf1) -> [256, 4]
  h3 = leaky_relu(Wf2^T @ hh + bf2)       -> [64, 4]
  o  = sigmoid(Wo^T @ h3 + bo)            -> [1, 4]

Sharding across the 8 NeuronCores: 4 graphs per core; nodes and edges
are partitioned by the graph of the edge *destination*, so aggregation,
pooling and the per-branch MLP are fully core-local.  X (bf16) and all
parameters are replicated.  Edges are dst-sorted and packed into
128-edge subtiles; gathers use indirect DMA on the src index, and the
scatter-add is a one-hot (norm-valued) matmul accumulated in PSUM.
The final [1, 4] per-core outputs are concatenated on the host.
"""

import contextlib

import numpy as np
import ml_dtypes

import concourse.bacc as bacc
import concourse.mybir as mybir
import concourse.tile as tile
from concourse.bass import IndirectOffsetOnAxis
from concourse.bass_utils import run_bass_kernel_spmd
from concourse.masks import make_identity

BF16 = mybir.dt.bfloat16
F32 = mybir.dt.float32
I32 = mybir.dt.int32
P = 128
N_CORES = 8
N_GRAPHS = 32
GPC = N_GRAPHS // N_CORES  # graphs per core

DIMS = dict(n_nodes=10000, f_in=1024, fp=128, hf1=256, hf2=64)


# ---------------------------------------------------------------- host prep


def _branch_prep(x, edge_index, batch, n_nodes, f_in):
    """Per-branch host preprocessing. Returns per-core arrays + static meta."""
    src = np.asarray(edge_index[0], dtype=np.int64)
    dst = np.asarray(edge_index[1], dtype=np.int64)
    batch = np.asarray(batch, dtype=np.int64)

    deg = np.bincount(dst, minlength=n_nodes).astype(np.float64) + 1.0
    dinv = (1.0 / np.sqrt(deg)).astype(np.float32).astype(np.float64)

    # append self loops
    allsrc = np.concatenate([src, np.arange(n_nodes, dtype=np.int64)])
    alldst = np.concatenate([dst, np.arange(n_nodes, dtype=np.int64)])
    norm = (dinv[allsrc] * dinv[alldst]).astype(np.float32)

    # node ranges per core (batch is sorted)
    bounds = np.searchsorted(batch, np.arange(0, N_GRAPHS + 1, GPC))
    n_per_core = np.diff(bounds)
    npad = int(np.ceil(max(1, n_per_core.max()) / P) * P)
    t_d = npad // P

    edge_core = batch[alldst] // GPC

    # per (core, dtile) edge counts -> shared T_s[d]
    e_counts = np.zeros((N_CORES, t_d), dtype=np.int64)
    core_edges = []
    for c in range(N_CORES):
        m = edge_core == c
        es, ed, en = allsrc[m], alldst[m], norm[m]
        ld = ed - bounds[c]
        order = np.argsort(ld, kind="stable")
        es, ld, en = es[order], ld[order], en[order]
        core_edges.append((es, ld, en))
        e_counts[c] = np.bincount(ld // P, minlength=t_d)
    t_s = np.maximum(1, np.ceil(e_counts.max(axis=0) / P).astype(np.int64))
    t0 = np.concatenate([[0], np.cumsum(t_s)])
    t_tot = int(t0[-1])

    x_bf = np.ascontiguousarray(np.asarray(x, dtype=np.float32)).astype(
        ml_dtypes.bfloat16
    )

    per_core = []
    for c in range(N_CORES):
        es, ld, en = core_edges[c]
        src_arr = np.zeros((t_tot, P), dtype=np.int32)
        hot = np.zeros((t_tot, P, P), dtype=ml_dtypes.bfloat16)
        for d in range(t_d):
            m = (ld // P) == d
            k = int(m.sum())
            if k == 0:
                continue
            sl_src = es[m]
            sl_m = (ld[m] - d * P).astype(np.int64)
            sl_n = en[m]
            for j in range(int(t_s[d])):
                lo, hi = j * P, min((j + 1) * P, k)
                if lo >= k:
                    break
                t = int(t0[d]) + j
                nrows = hi - lo
                src_arr[t, :nrows] = sl_src[lo:hi]
                hot[t, np.arange(nrows), sl_m[lo:hi]] = sl_n[lo:hi].astype(
                    ml_dtypes.bfloat16
                )
        # pool matrix [t_d, P, GPC]; counts
        pm = np.zeros((t_d, P, GPC), dtype=ml_dtypes.bfloat16)
        nc_lo, nc_hi = bounds[c], bounds[c + 1]
        loc_g = (batch[nc_lo:nc_hi] - c * GPC).astype(np.int64)
        idx = np.arange(nc_hi - nc_lo)
        pm[idx // P, idx % P, loc_g] = 1.0
        cnt = np.bincount(loc_g, minlength=GPC).astype(np.float64)
        ci = (1.0 / np.maximum(cnt, 1.0)).astype(np.float32)
        per_core.append(
            {
                # partition-major layouts so each dst-tile loads as one
                # contiguous-per-partition DMA
                "src": np.ascontiguousarray(src_arr.T),  # [P, t_tot]
                "hot": np.ascontiguousarray(
                    hot.transpose(1, 0, 2).reshape(P, t_tot * P)
                ),
                "pm": np.ascontiguousarray(
                    pm.transpose(1, 0, 2).reshape(P, t_d * GPC)
                ),
                "ci": np.broadcast_to(
                    np.tile(ci, f_in // P), (P, (f_in // P) * GPC)
                ).copy(),
            }
        )
    meta = {"t_d": t_d, "t_s": [int(v) for v in t_s], "t0": [int(v) for v in t0]}
    return x_bf, per_core, meta


def _ktile(w, f_in):
    """[f_in, n] -> [P, (f_in//P)*n] SBUF k-tile layout."""
    f, n = w.shape
    assert f == f_in
    return (
        np.ascontiguousarray(w)
        .reshape(f // P, P, n)
        .transpose(1, 0, 2)
        .reshape(P, (f // P) * n)
    )


def prep_inputs(inputs, dims):
    n_nodes, f_in = dims["n_nodes"], dims["f_in"]
    fp, hf1, hf2 = dims["fp"], dims["hf1"], dims["hf2"]

    x1_bf, pc1, meta1 = _branch_prep(
        inputs["pro1_x"], inputs["pro1_edge_index"], inputs["pro1_batch"], n_nodes, f_in
    )
    x2_bf, pc2, meta2 = _branch_prep(
        inputs["pro2_x"], inputs["pro2_edge_index"], inputs["pro2_batch"], n_nodes, f_in
    )

    f32 = np.float32
    shared = {
        "xg1": x1_bf,
        "xg2": x2_bf,
        "wg1": _ktile(np.asarray(inputs["Wg1"], f32), f_in).astype(ml_dtypes.bfloat16),
        "wg2": _ktile(np.asarray(inputs["Wg2"], f32), f_in).astype(ml_dtypes.bfloat16),
        "bg1": np.asarray(inputs["bg1"], f32)[None, :].astype(ml_dtypes.bfloat16),
        "bg2": np.asarray(inputs["bg2"], f32)[None, :].astype(ml_dtypes.bfloat16),
        "wp1": _ktile(np.asarray(inputs["Wp1"], f32), f_in),
        "wp2": _ktile(np.asarray(inputs["Wp2"], f32), f_in),
        "bp1": np.asarray(inputs["bp1"], f32)[:, None],
        "bp2": np.asarray(inputs["bp2"], f32)[:, None],
        "wf1": _ktile(np.asarray(inputs["Wf1"], f32), 2 * fp),
        "bf1": np.asarray(inputs["bf1"], f32).reshape(hf1 // P, P).T.copy(),
        "wf2": _ktile(np.asarray(inputs["Wf2"], f32), hf1),
        "bf2": np.asarray(inputs["bf2"], f32)[:, None],
        "wo": np.asarray(inputs["Wo"], f32),
        "bo": np.asarray(inputs["bo"], f32)[:, None],
    }
    in_maps = []
    for c in range(N_CORES):
        m = dict(shared)
        for br, pc in (("1", pc1), ("2", pc2)):
            for k in ("src", "hot", "pm", "ci"):
                m[k + br] = pc[c][k]
        in_maps.append(m)
    meta = {"b1": meta1, "b2": meta2, "dims": dims}
    return in_maps, meta


# ---------------------------------------------------------------- program


def _bias_leaky(nc, pool, out_ap, psum_ap, bias_col):
    """out = leaky_relu(psum + bias); bias_col is a per-partition [p,1] AP."""
    p, n = psum_ap.shape
    z = pool.tile([p, n], F32, tag="blz")
    nc.vector.tensor_scalar_add(out=z[:], in0=psum_ap, scalar1=bias_col)
    t = pool.tile([p, n], F32, tag="blt")
    nc.vector.tensor_scalar_mul(out=t[:], in0=z[:], scalar1=0.01)
    nc.vector.tensor_tensor(out=out_ap, in0=z[:], in1=t[:], op=mybir.AluOpType.max)


def build_program(meta, loop_n=1):
    dims = meta["dims"]
    n_nodes, f_in = dims["n_nodes"], dims["f_in"]
    fp, hf1, hf2 = dims["fp"], dims["hf1"], dims["hf2"]
    CH = f_in // P  # k-chunks of gcn layer
    NH = (f_in + 511) // 512  # N-halves of 512
    NS = min(f_in, 512)

    nc = bacc.Bacc("TRN2", target_bir_lowering=False, debug=False, num_devices=N_CORES)

    def din(name, shape, dt):
        return nc.dram_tensor(name, list(shape), dt, kind="ExternalInput").ap()

    aps = {}
    for br in ("1", "2"):
        m = meta["b" + br]
        t_tot = m["t0"][-1]
        aps["xg" + br] = din("xg" + br, [n_nodes, f_in], BF16)
        aps["src" + br] = din("src" + br, [P, t_tot], I32)
        aps["hot" + br] = din("hot" + br, [P, t_tot * P], BF16)
        aps["pm" + br] = din("pm" + br, [P, m["t_d"] * GPC], BF16)
        aps["ci" + br] = din("ci" + br, [P, CH * GPC], F32)
        aps["wg" + br] = din("wg" + br, [P, CH * f_in], BF16)
        aps["bg" + br] = din("bg" + br, [1, f_in], BF16)
        aps["wp" + br] = din("wp" + br, [P, CH * fp], F32)
        aps["bp" + br] = din("bp" + br, [fp, 1], F32)
    aps["wf1"] = din("wf1", [P, (2 * fp // P) * hf1], F32)
    aps["bf1"] = din("bf1", [P, hf1 // P], F32)
    aps["wf2"] = din("wf2", [P, (hf1 // P) * hf2], F32)
    aps["bf2"] = din("bf2", [hf2, 1], F32)
    aps["wo"] = din("wo", [hf2, 1], F32)
    aps["bo"] = din("bo", [1, 1], F32)
    out_ap = nc.dram_tensor("out", [1, GPC], F32, kind="ExternalOutput").ap()

    SIG = mybir.ActivationFunctionType.Sigmoid

    with tile.TileContext(nc) as tc:
        with (
            tc.tile_pool(name="const", bufs=1) as cpool,
            tc.tile_pool(name="gp", bufs=3) as gpool,
            tc.tile_pool(name="hp", bufs=3) as hpool,
            tc.tile_pool(name="ip", bufs=6) as ipool,
            tc.tile_pool(name="sp", bufs=2) as spool,
            tc.tile_pool(name="tp", bufs=2) as tpool,
            tc.tile_pool(name="lp", bufs=2) as lpool,
            tc.tile_pool(name="acc", bufs=1) as apool,
            tc.tile_pool(name="spsum", bufs=2, space="PSUM") as spsum,
            tc.tile_pool(name="tpsum", bufs=1, space="PSUM") as tpsum,
            tc.tile_pool(name="cpsum", bufs=1, space="PSUM") as cpsum,
            tc.tile_pool(name="mpsum", bufs=1, space="PSUM") as mpsum,
        ):
            ident = cpool.tile([P, P], BF16)
            make_identity(nc, ident[:])
            ones1 = cpool.tile([1, P], BF16)
            nc.vector.memset(ones1[:], 1.0)

            # persistent weights
            wt = {}
            for name, dt in (
                ("wg1", BF16), ("wg2", BF16), ("bg1", BF16), ("bg2", BF16),
                ("wp1", F32), ("wp2", F32), ("bp1", F32), ("bp2", F32),
                ("ci1", F32), ("ci2", F32),
                ("wf1", F32), ("bf1", F32), ("wf2", F32), ("bf2", F32),
                ("wo", F32), ("bo", F32),
            ):
                t = cpool.tile(list(aps[name].shape), dt, tag=name)
                nc.sync.dma_start(out=t[:], in_=aps[name][:])
                wt[name] = t

            def emit_body():
                hbr = {}
                for br in ("1", "2"):
                    m = meta["b" + br]
                    t_d, t_s, t0 = m["t_d"], m["t_s"], m["t0"]
                    xg, srca, hota, pma = (
                        aps["xg" + br], aps["src" + br], aps["hot" + br],
                        aps["pm" + br],
                    )
                    poolacc = apool.tile([P, CH * GPC], F32, tag="poolacc" + br)
                    nc.vector.memset(poolacc[:], 0.0)
                    pmt = hpool.tile([P, t_d * GPC], BF16, tag="pm")
                    nc.sync.dma_start(out=pmt[:], in_=pma[:])

                    for d in range(t_d):
                        ts, td0 = t_s[d], t0[d]
                        s_ps = spsum.tile([P, f_in], F32, tag="s")
                        idxt = ipool.tile([P, ts], I32, tag="idx")
                        nc.sync.dma_start(
                            out=idxt[:], in_=srca[:, td0 : td0 + ts]
                        )
                        hott = hpool.tile([P, ts * P], BF16, tag="hot")
                        nc.sync.dma_start(
                            out=hott[:], in_=hota[:, td0 * P : (td0 + ts) * P]
                        )
                        GB = 4  # subtiles gathered per indirect DMA
                        for j0 in range(0, ts, GB):
                            gn = min(GB, ts - j0)
                            g = gpool.tile([P, GB * f_in], BF16, tag="g")
                            nc.gpsimd.indirect_dma_start(
                                out=g[:, : gn * f_in],
                                out_offset=None,
                                in_=xg[:],
                                in_offset=IndirectOffsetOnAxis(
                                    ap=idxt[:, j0 : j0 + gn], axis=0
                                ),
                            )
                            for jj in range(gn):
                                j = j0 + jj
                                for h in range(NH):
                                    nc.tensor.matmul(
                                        s_ps[:, h * NS : (h + 1) * NS],
                                        lhsT=hott[:, j * P : (j + 1) * P],
                                        rhs=g[:, jj * f_in + h * NS : jj * f_in + (h + 1) * NS],
                                        start=(j == 0),
                                        stop=(j == ts - 1),
                                    )
                        s_sb = spool.tile([P, f_in], BF16, tag="s_sb")
                        nc.scalar.copy(out=s_sb[:], in_=s_ps[:])
                        t_ps = tpsum.tile([P, f_in], BF16, tag="t_ps")
                        for ck in range(CH):
                            nc.tensor.transpose(
                                t_ps[:, ck * P : (ck + 1) * P],
                                s_sb[:, ck * P : (ck + 1) * P],
                                ident[:],
                            )
                        t_sb = tpool.tile([P, f_in], BF16, tag="t_sb")
                        nc.vector.tensor_copy(out=t_sb[:], in_=t_ps[:])

                        c_ps = cpsum.tile([P, f_in], F32, tag="c_ps")
                        for h in range(NH):
                            for kk in range(CH):
                                nc.tensor.matmul(
                                    c_ps[:, h * NS : (h + 1) * NS],
                                    lhsT=t_sb[:, kk * P : (kk + 1) * P],
                                    rhs=wt["wg" + br][
                                        :, kk * f_in + h * NS : kk * f_in + (h + 1) * NS
                                    ],
                                    start=(kk == 0),
                                    stop=False,
                                )
                            nc.tensor.matmul(
                                c_ps[:, h * NS : (h + 1) * NS],
                                lhsT=ones1[:1, :],
                                rhs=wt["bg" + br][:1, h * NS : (h + 1) * NS],
                                start=False,
                                stop=True,
                            )
                        leak = lpool.tile([P, f_in], BF16, tag="leak")
                        lk01 = lpool.tile([P, f_in], F32, tag="lk01")
                        nc.vector.tensor_scalar_mul(
                            out=lk01[:], in0=c_ps[:], scalar1=0.01
                        )
                        nc.vector.tensor_tensor(
                            out=leak[:], in0=c_ps[:], in1=lk01[:],
                            op=mybir.AluOpType.max,
                        )

                        p_ps = mpsum.tile([P, CH * GPC], F32, tag="small")
                        for ck in range(CH):
                            nc.tensor.matmul(
                                p_ps[:, ck * GPC : (ck + 1) * GPC],
                                lhsT=leak[:, ck * P : (ck + 1) * P],
                                rhs=pmt[:, d * GPC : (d + 1) * GPC],
                                start=True,
                                stop=True,
                            )
                        nc.vector.tensor_add(
                            out=poolacc[:], in0=poolacc[:], in1=p_ps[:]
                        )

                    # scale by 1/cnt, then h = lrelu(Wp^T @ pool + bp)
                    nc.vector.tensor_tensor(
                        out=poolacc[:],
                        in0=poolacc[:],
                        in1=wt["ci" + br][:],
                        op=mybir.AluOpType.mult,
                    )
                    h_ps = mpsum.tile([P, GPC], F32, tag="small")
                    for ck in range(CH):
                        nc.tensor.matmul(
                            h_ps[:, :],
                            lhsT=wt["wp" + br][:, ck * fp : (ck + 1) * fp],
                            rhs=poolacc[:, ck * GPC : (ck + 1) * GPC],
                            start=(ck == 0),
                            stop=(ck == CH - 1),
                        )
                    hb = apool.tile([fp, GPC], F32, tag="hbr" + br)
                    _bias_leaky(nc, apool, hb[:], h_ps[:fp, :], wt["bp" + br][:, :1])
                    hbr[br] = hb

                # head
                K1 = 2 * fp // P
                M1 = hf1 // P
                rhs_k = [hbr["1"], hbr["2"]]
                hh = apool.tile([P, M1 * GPC], F32, tag="hh")
                for mt in range(M1):
                    f_ps = mpsum.tile([P, GPC], F32, tag="small")
                    for kk in range(K1):
                        nc.tensor.matmul(
                            f_ps[:, :],
                            lhsT=wt["wf1"][
                                :, kk * hf1 + mt * P : kk * hf1 + (mt + 1) * P
                            ],
                            rhs=rhs_k[kk][:, :],
                            start=(kk == 0),
                            stop=(kk == K1 - 1),
                        )
                    _bias_leaky(
                        nc, apool, hh[:, mt * GPC : (mt + 1) * GPC], f_ps[:, :],
                        wt["bf1"][:, mt : mt + 1],
                    )
                g_ps = mpsum.tile([hf2, GPC], F32, tag="small")
                for kk in range(M1):
                    nc.tensor.matmul(
                        g_ps[:, :],
                        lhsT=wt["wf2"][:, kk * hf2 : (kk + 1) * hf2],
                        rhs=hh[:, kk * GPC : (kk + 1) * GPC],
                        start=(kk == 0),
                        stop=(kk == M1 - 1),
                    )
                h3 = apool.tile([hf2, GPC], F32, tag="h3")
                _bias_leaky(nc, apool, h3[:], g_ps[:], wt["bf2"][:, :1])
                o_ps = mpsum.tile([1, GPC], F32, tag="small")
                nc.tensor.matmul(
                    o_ps[:, :], lhsT=wt["wo"][:, :1], rhs=h3[:, :],
                    start=True, stop=True,
                )
                o_sb = apool.tile([1, GPC], F32, tag="o_sb")
                nc.scalar.activation(
                    out=o_sb[:], in_=o_ps[:], func=SIG, bias=wt["bo"][:1, :1]
                )
                nc.sync.dma_start(out=out_ap[:], in_=o_sb[:])

            if loop_n > 1:
                with tc.For_i(0, loop_n, 1):
                    emit_body()
            else:
                emit_body()

    nc.compile()
    return nc


# ---------------------------------------------------------------- entry


_CACHE = {}


def _program_key(meta):
    return (
        tuple(meta["b1"]["t_s"]),
        tuple(meta["b2"]["t_s"]),
        meta["b1"]["t_d"],
        meta["b2"]["t_d"],
    )


def get_program(meta):
    key = _program_key(meta)
    if key not in _CACHE:
        _CACHE[key] = build_program(meta)
    return _CACHE[key]


def kernel(**inputs) -> np.ndarray:
    in_maps, meta = prep_inputs(inputs, DIMS)
    nc = get_program(meta)
    res = run_bass_kernel_spmd(nc, in_maps, core_ids=list(range(N_CORES)))
    out = np.concatenate(
        [
            np.asarray(res.results[c]["out"], dtype=np.float32).reshape(GPC)
            for c in range(N_CORES)
        ]
    )
    return out[:, None]
